# revision 1
# baseline (speedup 1.0000x reference)
"""Causal single-head attention (b=4, n=2048, d=1024) on 8 trn2 cores.

Sharding: 2 cores per batch element. Each batch's 16 query blocks (128
rows) are assigned to its core pair so that every core processes one
q-block at each "capacity" in {2,4,...,16} key-blocks: even-parity
cores take even-index q-blocks (odd causal limit), odd-parity cores
take odd-index ones (even causal limit). Odd causal limits waste one
fully-masked 128-key block; total per-core key-block visits = 72
(vs 68 ideal) and the instruction stream is identical on all cores
(pure SPMD) — only the data (gathered q rows + mask) differs.

Per core: K^T/V/Q^T projections (PE), scores = Q^T·K per q-block,
masked softmax (DVE reduce + ACT exp), PE transpose of the weights,
AV accumulation, 1/rowsum folded into the PSUM->SBUF copyback.
The 1/sqrt(d) score scale (2^-5, exact) is folded into Q^T.
"""

import numpy as np

P = 128
B, N, D = 4, 2048, 1024
NCORES = 8
CAPS = (16, 14, 12, 10, 8, 6, 4, 2)  # key-block capacity per slot
NEG = -1.0e30

# Matmul compute dtype: "f32" (exact, 4 cyc/row) or "f32r" (full rate,
# TF32-ish hardware numerics).
MM_DT = "f32r"

_prog_cache = {}


def _split_multi_waits(nc, max_waits=1):
    """walrus in this container rejects more than one sem wait per
    instruction ("Too many sync wait commands"). After Tile scheduling,
    hoist extra waits onto same-engine nops inserted just before the
    instruction (same blocking semantics: engine queues are in-order)."""
    from concourse import mybir

    n = 0
    for fn in nc.m.functions:
        for bb in fn.blocks:
            out = []
            for ins in bb.instructions:
                si = ins.sync_info
                waits = list(si.on_wait) if si and si.on_wait else []
                if len(waits) > max_waits:
                    extra = waits[:-max_waits]
                    si.on_wait = waits[-max_waits:]
                    for j in range(0, len(extra), max_waits):
                        nop = mybir.InstNoOp(
                            name=f"waitsplit_{n}", ins=[], outs=[],
                            engine=ins.engine)
                        n += 1
                        nop.sync_info = mybir.SyncInfo(
                            on_wait=extra[j:j + max_waits], on_update=[])
                        out.append(nop)
                out.append(ins)
            bb.instructions[:] = out


def _build_program(mm_dt_name):
    import concourse.bass as bass
    import concourse.tile as tile
    from concourse import mybir
    from concourse.masks import make_identity

    f32 = mybir.dt.float32
    mmdt = f32 if mm_dt_name == "f32" else mybir.dt.float32r

    nc = bass.Bass("TRN2", target_bir_lowering=False, debug=False,
                   num_devices=NCORES, dynamic_dma_scratch_size=2048)

    xqT_d = nc.dram_tensor("xqT", [D, 8 * P], mmdt, kind="ExternalInput").ap()
    xkT_d = nc.dram_tensor("xkT", [D, N], mmdt, kind="ExternalInput").ap()
    wq_d = nc.dram_tensor("wq", [D, D], mmdt, kind="ExternalInput").ap()
    wk_d = nc.dram_tensor("wk", [D, D], mmdt, kind="ExternalInput").ap()
    wv_d = nc.dram_tensor("wv", [D, D], mmdt, kind="ExternalInput").ap()
    mask_d = nc.dram_tensor("mask", [P, 2 * P], f32, kind="ExternalInput").ap()
    out_d = nc.dram_tensor("out", [8 * P, D], f32, kind="ExternalOutput").ap()

    DC = D // P  # 8 contraction chunks
    xqT_r = xqT_d.rearrange("(dc p) q -> p dc q", p=P)
    xkT_r = xkT_d.rearrange("(dc p) k -> p dc k", p=P)
    wq_r = wq_d.rearrange("(dc p) e -> p dc e", p=P)
    wk_r = wk_d.rearrange("(dc p) e -> p dc e", p=P)
    wv_r = wv_d.rearrange("(dc p) e -> p dc e", p=P)

    with tile.TileContext(nc) as tc:
        import contextlib
        with contextlib.ExitStack() as ctx:
            cpool = ctx.enter_context(tc.tile_pool(name="cpool", bufs=1))
            qtp = ctx.enter_context(tc.tile_pool(name="qtp", bufs=1))
            ktp = ctx.enter_context(tc.tile_pool(name="ktp", bufs=1))
            vp = ctx.enter_context(tc.tile_pool(name="vp", bufs=1))

            ident_f = cpool.tile([P, P], f32, name="ident_f")
            make_identity(nc, ident_f)
            ident = cpool.tile([P, P], mmdt, name="ident")
            nc.vector.tensor_copy(ident[:], ident_f[:])
            mask_sb = cpool.tile([P, 2 * P], f32, name="mask_sb")
            nc.sync.dma_start(mask_sb[:], mask_d)

            QT = qtp.tile([P, DC, 8 * P], mmdt, name="QT")
            KT = ktp.tile([P, DC, N], mmdt, name="KT")
            V = vp.tile([P, N // P, D], mmdt, name="V")

            # ---- projections ----
            # Weights stream as four [P, 2, D] quarters (8KB/partition)
            # through 5 shared slots so the next phase's weights prefetch
            # into free slots while the current phase computes.
            with tc.tile_pool(name="wpool", bufs=5) as wpool, \
                 tc.tile_pool(name="xpool", bufs=2) as xpool, \
                 tc.tile_pool(name="ppj", bufs=4, space="PSUM") as ppj:

                def load_w(src_r, nm):
                    qs = []
                    for i in range(4):
                        t = wpool.tile([P, 2, D], mmdt, tag="w",
                                       name=f"{nm}_q{i}")
                        nc.sync.dma_start(t[:], src_r[:, 2 * i:2 * i + 2, :])
                        qs.append(t)
                    return qs

                # Q^T[e, q] = sum_d Wq[d, e] * xqT[d, q], scaled by 1/32
                # first x slice is DMA'd before the weights so the PE can
                # start as soon as the first weight quarters land
                xs0 = xpool.tile([P, DC, 256], mmdt, tag="xs", name="xs_q0")
                nc.sync.dma_start(xs0[:], xqT_r[:, :, 0:256])
                wq2 = load_w(wq_r, "wq")
                for qt in range(4):
                    if qt == 0:
                        xs = xs0
                    else:
                        xs = xpool.tile([P, DC, 256], mmdt, tag="xs",
                                        name="xs_q")
                        nc.sync.dma_start(
                            xs[:], xqT_r[:, :, qt * 256:(qt + 1) * 256])
                    for ec in range(DC):
                        ps = ppj.tile([P, 512], f32, tag="pj", name="ps_q")
                        for dc in range(DC):
                            nc.tensor.matmul(
                                ps[:, :256],
                                wq2[dc // 2][:, dc % 2, ec * P:(ec + 1) * P],
                                xs[:, dc, :],
                                start=(dc == 0), stop=(dc == DC - 1))
                        nc.vector.tensor_scalar_mul(
                            QT[:, ec, qt * 256:(qt + 1) * 256],
                            ps[:, :256], 1.0 / 32.0)

                # K^T[e, k] = sum_d Wk[d, e] * xkT[d, k]
                wk2 = load_w(wk_r, "wk")
                for kt in range(8):
                    xs = xpool.tile([P, DC, 256], mmdt, tag="xs", name="xs_k")
                    nc.sync.dma_start(xs[:], xkT_r[:, :, kt * 256:(kt + 1) * 256])
                    for ec in range(DC):
                        ps = ppj.tile([P, 512], f32, tag="pj", name="ps_k")
                        for dc in range(DC):
                            nc.tensor.matmul(
                                ps[:, :256],
                                wk2[dc // 2][:, dc % 2, ec * P:(ec + 1) * P],
                                xs[:, dc, :],
                                start=(dc == 0), stop=(dc == DC - 1))
                        nc.vector.tensor_copy(
                            KT[:, ec, kt * 256:(kt + 1) * 256], ps[:, :256])

                # V[k, e] = sum_d xkT[d, k] * Wv[d, e]
                wv2 = load_w(wv_r, "wv")
                for kp in range(N // 256):
                    xs = xpool.tile([P, DC, 256], mmdt, tag="xs", name="xs_v")
                    nc.sync.dma_start(xs[:], xkT_r[:, :, kp * 256:(kp + 1) * 256])
                    for half in range(2):
                        kc = 2 * kp + half
                        for h in range(2):
                            ps = ppj.tile([P, 512], f32, tag="pj", name="ps_v")
                            for dc in range(DC):
                                nc.tensor.matmul(
                                    ps,
                                    xs[:, dc, half * P:(half + 1) * P],
                                    wv2[dc // 2][:, dc % 2,
                                                 h * 512:(h + 1) * 512],
                                    start=(dc == 0), stop=(dc == DC - 1))
                            nc.vector.tensor_copy(
                                V[:, kc, h * 512:(h + 1) * 512], ps)

            # ---- attention, software-pipelined over the 8 slots ----
            with tc.tile_pool(name="scp", bufs=3) as scp, \
                 tc.tile_pool(name="wtp", bufs=2) as wtp, \
                 tc.tile_pool(name="obp", bufs=2) as obp, \
                 tc.tile_pool(name="stp", bufs=3) as stp, \
                 tc.tile_pool(name="psc", bufs=2, space="PSUM") as psc, \
                 tc.tile_pool(name="pav", bufs=4, space="PSUM") as pav, \
                 tc.tile_pool(name="ptr", bufs=2, space="PSUM") as ptr:

                scores = [None] * len(CAPS)
                stats = [None] * len(CAPS)

                def emit_scores(slot):
                    s = CAPS[slot]
                    L = P * s
                    sc = scp.tile([P, N], mmdt, tag="sc", name=f"sc{slot}")
                    st = stp.tile([P, 4], f32, tag="st", name=f"st{slot}")
                    scores[slot] = sc
                    stats[slot] = st
                    off = 0
                    widths = [512] * (L // 512) + ([256] if L % 512 else [])
                    for w in widths:
                        ps = psc.tile([P, 512], f32, tag="psc", name=f"pssc{slot}")
                        for ec in range(DC):
                            nc.tensor.matmul(
                                ps[:, :w],
                                QT[:, ec, slot * P:(slot + 1) * P],
                                KT[:, ec, off:off + w],
                                start=(ec == 0), stop=(ec == DC - 1))
                        end = off + w
                        if end == L:
                            if w == 512:
                                nc.vector.tensor_copy(
                                    sc[:, off:off + 256], ps[:, 0:256])
                            nc.vector.tensor_add(
                                sc[:, L - 256:L], ps[:, w - 256:w], mask_sb[:])
                        else:
                            nc.vector.tensor_copy(sc[:, off:end], ps[:, :w])
                        off = end
                    # softmax stats + in-place exp
                    nc.vector.tensor_reduce(
                        st[:, 0:1], sc[:, :L], axis=mybir.AxisListType.X,
                        op=mybir.AluOpType.max, negate=True)
                    nc.scalar.activation(
                        sc[:, :L], sc[:, :L], mybir.ActivationFunctionType.Exp,
                        bias=st[:, 0:1], scale=1.0, accum_out=st[:, 1:2])
                    nc.vector.reciprocal(st[:, 2:3], st[:, 1:2])

                def emit_av(slot):
                    s = CAPS[slot]
                    sc = scores[slot]
                    st = stats[slot]
                    wt = wtp.tile([P, N // P, P], mmdt, tag="wt", name=f"wt{slot}")
                    for j in range(s):
                        pt = ptr.tile([P, P], mmdt, tag="ptr", name=f"pt{slot}")
                        nc.tensor.transpose(pt, sc[:, j * P:(j + 1) * P], ident)
                        nc.vector.tensor_copy(wt[:, j, :], pt)
                    avs = []
                    for h in range(2):
                        av = pav.tile([P, 512], f32, tag="pav", name=f"av{slot}_{h}")
                        avs.append(av)
                    for j in range(s):
                        for h in range(2):
                            nc.tensor.matmul(
                                avs[h],
                                wt[:, j, :],
                                V[:, j, h * 512:(h + 1) * 512],
                                start=(j == 0), stop=(j == s - 1))
                    ob = obp.tile([P, D], f32, tag="ob", name=f"ob{slot}")
                    for h in range(2):
                        nc.vector.tensor_scalar_mul(
                            ob[:, h * 512:(h + 1) * 512], avs[h], st[:, 2:3])
                    nc.sync.dma_start(out_d[slot * P:(slot + 1) * P, :], ob)

                emit_scores(0)
                emit_scores(1)
                for b_ in range(len(CAPS)):
                    if b_ + 2 < len(CAPS):
                        emit_scores(b_ + 2)
                    emit_av(b_)

    _split_multi_waits(nc)
    return nc


def _host_prep(x, Wq, Wk, Wv):
    """Build per-core input maps."""
    x = np.ascontiguousarray(x, dtype=np.float32)
    tri = np.where(
        np.arange(P)[None, :] <= np.arange(P)[:, None], 0.0, NEG
    ).astype(np.float32)
    mask_even = np.concatenate(  # parity 0: diag block then fully-masked block
        [tri, np.full((P, P), NEG, np.float32)], axis=1)
    mask_odd = np.concatenate(  # parity 1: fully-visible block then diag block
        [np.zeros((P, P), np.float32), tri], axis=1)

    in_maps = []
    for c in range(NCORES):
        bi, r = c // 2, c % 2
        rbs = [s - 2 + r for s in CAPS]
        xq = np.concatenate([x[bi, rb * P:(rb + 1) * P, :] for rb in rbs], axis=0)
        in_maps.append({
            "xqT": np.ascontiguousarray(xq.T),
            "xkT": np.ascontiguousarray(x[bi].T),
            "wq": np.ascontiguousarray(Wq, dtype=np.float32),
            "wk": np.ascontiguousarray(Wk, dtype=np.float32),
            "wv": np.ascontiguousarray(Wv, dtype=np.float32),
            "mask": mask_odd if r else mask_even,
        })
    return in_maps


def _host_gather(results):
    out = np.empty((B, N, D), dtype=np.float32)
    for c in range(NCORES):
        bi, r = c // 2, c % 2
        res = results[c]["out"]
        for k, s in enumerate(CAPS):
            rb = s - 2 + r
            out[bi, rb * P:(rb + 1) * P, :] = res[k * P:(k + 1) * P, :]
    return out


def kernel(x, Wq, Wk, Wv, _trace=False, _trace_kwargs=None):
    from concourse.bass_utils import run_bass_kernel_spmd

    key = MM_DT
    if key not in _prog_cache:
        _prog_cache[key] = _build_program(key)
    nc = _prog_cache[key]

    in_maps = _host_prep(x, Wq, Wk, Wv)
    kw = dict(_trace_kwargs or {})
    res = run_bass_kernel_spmd(nc, in_maps, list(range(NCORES)),
                               trace=_trace, **kw)
    out = _host_gather(res.results)
    if _trace:
        return out, res
    return out



# revision 3
# speedup vs baseline: 3.6558x; 3.6558x over previous
"""Causal single-head attention (b=4, n=2048, d=1024) on 8 trn2 cores.

Sharding: 2 cores per batch element (pairs [0,1],[2,3],[4,5],[6,7]).
Within a pair, the K^T and V projections are split along the output
(d_out) dimension: rank r of the pair computes only e in
[512r, 512r+512), and the halves are exchanged with a pairwise
AllGather through DRAM bounce buffers (a tiny warmup AllGather at t=0
absorbs the collective entry latency). Q^T is computed locally for the
core's own 8 query blocks (full e). This removes the duplicated K/V
projection work of the pure data-parallel layout.

Each batch's 16 query blocks (128 rows) are split by parity so every
core processes one q-block at each "capacity" in {2,4,...,16}
key-blocks; the instruction stream is identical on all cores (pure
SPMD) - only the data differs.

Compute is bf16 on the PE (tolerance 2e-2; measured ~2e-3): K/V/Q
projections, scores = Q^T.K per q-block, exp directly from PSUM on the
Scalar engine (no max subtraction - logits are O(5), safe in f32) with
fused row-sum accumulation, PE transpose of the exp'd weights, then
all AV matmuls emitted after all scores so the V AllGather only has to
land late. 1/sqrt(d) = 2^-5 is folded into Q^T; 1/rowsum is folded
into the PSUM->SBUF copyback of AV.
"""

import numpy as np

P = 128
B, N, D = 4, 2048, 1024
NCORES = 8
CAPS = (16, 14, 12, 10, 8, 6, 4, 2)  # key-block capacity per slot
SUMCAPS = sum(CAPS)  # 72 key-block visits per core
NEG = -1.0e30
PAIRS = [[0, 1], [2, 3], [4, 5], [6, 7]]
EH = D // 2  # 512: e-columns computed locally per core for K/V

MM_DT = "bf16"  # informational; test.py prints it

_prog_cache = {}


def _split_multi_waits(nc, max_waits=1):
    """walrus in this container rejects more than one sem wait per
    instruction ("Too many sync wait commands"). After Tile scheduling,
    hoist extra waits onto same-engine nops inserted just before the
    instruction (same blocking semantics: engine queues are in-order)."""
    from concourse import mybir

    n = 0
    for fn in nc.m.functions:
        for bb in fn.blocks:
            out = []
            for ins in bb.instructions:
                si = ins.sync_info
                waits = list(si.on_wait) if si and si.on_wait else []
                if len(waits) > max_waits:
                    extra = waits[:-max_waits]
                    si.on_wait = waits[-max_waits:]
                    for j in range(0, len(extra), max_waits):
                        nop = mybir.InstNoOp(
                            name=f"waitsplit_{n}", ins=[], outs=[],
                            engine=ins.engine)
                        n += 1
                        nop.sync_info = mybir.SyncInfo(
                            on_wait=extra[j:j + max_waits], on_update=[])
                        out.append(nop)
                out.append(ins)
            bb.instructions[:] = out


def _build_program():
    import concourse.bass as bass
    import concourse.tile as tile
    from concourse import mybir
    from concourse.masks import make_identity

    f32 = mybir.dt.float32
    bf16 = mybir.dt.bfloat16
    DC = D // P  # 8 contraction chunks

    nc = bass.Bass("TRN2", target_bir_lowering=False, debug=False,
                   num_devices=NCORES, dynamic_dma_scratch_size=2048)

    xT_d = nc.dram_tensor("xT", [D, N], bf16, kind="ExternalInput").ap()
    xqT_d = nc.dram_tensor("xqT", [D, 8 * P], bf16, kind="ExternalInput").ap()
    wq_d = nc.dram_tensor("wq", [D, D], bf16, kind="ExternalInput").ap()
    wkh_d = nc.dram_tensor("wkh", [D, EH], bf16, kind="ExternalInput").ap()
    wvh_d = nc.dram_tensor("wvh", [D, EH], bf16, kind="ExternalInput").ap()
    mask_d = nc.dram_tensor("mask", [P, 2 * P], f32, kind="ExternalInput").ap()
    out_d = nc.dram_tensor("out", [8 * P, D], f32, kind="ExternalOutput").ap()

    # AllGather bounce buffers (pairwise, rank-major gather along dim 0)
    warm_i = nc.dram_tensor("warm_i", [P, 16], bf16, kind="Internal").ap()
    warm_o = nc.dram_tensor("warm_o", [2, P, 16], bf16, kind="Internal").ap()
    kag_i = nc.dram_tensor("kag_i", [4, P, N], bf16, kind="Internal").ap()
    kag_o = nc.dram_tensor("kag_o", [8, P, N], bf16, kind="Internal").ap()
    vag_i = nc.dram_tensor("vag_i", [N // P, P, EH], bf16, kind="Internal").ap()
    vag_o = nc.dram_tensor("vag_o", [2, N // P, P, EH], bf16,
                           kind="Internal").ap()

    xT_r = xT_d.rearrange("(dc p) k -> p dc k", p=P)
    xqT_r = xqT_d.rearrange("(dc p) q -> p dc q", p=P)
    wq_r = wq_d.rearrange("(dc p) e -> p dc e", p=P)
    wkh_r = wkh_d.rearrange("(dc p) e -> p dc e", p=P)
    wvh_r = wvh_d.rearrange("(dc p) e -> p dc e", p=P)

    offs = [128 * sum(CAPS[:s]) for s in range(len(CAPS))]  # sc/wt offsets

    def ag(ins_ap, outs_ap):
        nc.gpsimd.collective_compute(
            "AllGather", mybir.AluOpType.bypass, replica_groups=PAIRS,
            ins=[ins_ap], outs=[outs_ap])

    with tile.TileContext(nc) as tc:
        import contextlib
        with contextlib.ExitStack() as ctx:
            cpool = ctx.enter_context(tc.tile_pool(name="cpool", bufs=1))
            qtp = ctx.enter_context(tc.tile_pool(name="qtp", bufs=1))
            ktp = ctx.enter_context(tc.tile_pool(name="ktp", bufs=1))
            vp = ctx.enter_context(tc.tile_pool(name="vp", bufs=1))
            scp = ctx.enter_context(tc.tile_pool(name="scp", bufs=1))
            wtp = ctx.enter_context(tc.tile_pool(name="wtp", bufs=1))
            stp = ctx.enter_context(tc.tile_pool(name="stp", bufs=1))

            # ---- warmup AllGather: absorbs collective entry latency ----
            warm = cpool.tile([P, 16], bf16, name="warm")
            nc.vector.memset(warm[:], 0.0)
            nc.sync.dma_start(warm_i, warm[:])
            ag(warm_i, warm_o)

            ident_f = cpool.tile([P, P], f32, name="ident_f")
            make_identity(nc, ident_f)
            ident = cpool.tile([P, P], bf16, name="ident")
            nc.vector.tensor_copy(ident[:], ident_f[:])
            mask_sb = cpool.tile([P, 2 * P], f32, name="mask_sb")
            nc.sync.dma_start(mask_sb[:], mask_d)

            QT = qtp.tile([P, DC, 8 * P], bf16, name="QT")
            KT = ktp.tile([P, DC, N], bf16, name="KT")
            V = vp.tile([P, N // P, D], bf16, name="V")
            SC = scp.tile([P, SUMCAPS * P], bf16, name="SC")  # exp'd scores
            WT = wtp.tile([P, SUMCAPS * P], bf16, name="WT")  # transposed
            ST = stp.tile([P, len(CAPS), 6], f32, name="ST")  # rowsum stats

            # ---- projections ----
            with tc.tile_pool(name="wpool", bufs=1) as wpool, \
                 tc.tile_pool(name="xpool", bufs=2) as xpool, \
                 tc.tile_pool(name="cst", bufs=3) as cst, \
                 tc.tile_pool(name="ppj", bufs=4, space="PSUM") as ppj:

                wkh = wpool.tile([P, DC, EH], bf16, name="wkh")
                nc.sync.dma_start(wkh[:], wkh_r)
                wvh = wpool.tile([P, DC, EH], bf16, name="wvh")
                wq = wpool.tile([P, DC, D], bf16, name="wq")

                # K^T e-half: K^T[512r + ec*128 + p, k] for local ec 0..3
                for c in range(4):
                    xs = xpool.tile([P, DC, 512], bf16, tag="xs", name="xs_k")
                    nc.sync.dma_start(xs[:], xT_r[:, :, c * 512:(c + 1) * 512])
                    for ec in range(4):
                        ps = ppj.tile([P, 512], f32, tag="pj", name="ps_k")
                        for dc in range(DC):
                            nc.tensor.matmul(
                                ps,
                                wkh[:, dc, ec * P:(ec + 1) * P],
                                xs[:, dc, :],
                                start=(dc == 0), stop=(dc == DC - 1))
                        kst = cst.tile([P, 512], bf16, tag="cst", name="kst")
                        nc.vector.tensor_copy(kst[:], ps)
                        nc.sync.dma_start(
                            kag_i[ec, :, c * 512:(c + 1) * 512], kst[:])
                ag(kag_i, kag_o)
                # prefetch remaining weights while AG_K flies
                nc.sync.dma_start(wvh[:], wvh_r)
                nc.sync.dma_start(wq[:], wq_r)

                # V e-half: V[k, 512r + j] for j in 0..511
                for c in range(4):
                    xs = xpool.tile([P, DC, 512], bf16, tag="xs", name="xs_v")
                    nc.sync.dma_start(xs[:], xT_r[:, :, c * 512:(c + 1) * 512])
                    for kb in range(4):
                        ps = ppj.tile([P, 512], f32, tag="pj", name="ps_v")
                        for dc in range(DC):
                            nc.tensor.matmul(
                                ps,
                                xs[:, dc, kb * P:(kb + 1) * P],
                                wvh[:, dc, :],
                                start=(dc == 0), stop=(dc == DC - 1))
                        vst = cst.tile([P, 512], bf16, tag="cst", name="vst")
                        nc.vector.tensor_copy(vst[:], ps)
                        nc.sync.dma_start(vag_i[4 * c + kb], vst[:])
                ag(vag_i, vag_o)

                # AllGather readbacks (scalar engine queue so later sync
                # DMAs don't queue behind the AG-completion waits)
                nc.scalar.dma_start(
                    KT[:], kag_o.rearrange("eg p k -> p eg k"))
                nc.scalar.dma_start(
                    V[:, :, 0:EH],
                    vag_o[0].rearrange("kc p e -> p kc e"))
                nc.scalar.dma_start(
                    V[:, :, EH:D],
                    vag_o[1].rearrange("kc p e -> p kc e"))

                # Q^T full e for own q rows, scaled by 1/32
                for qc in range(2):
                    xs = xpool.tile([P, DC, 512], bf16, tag="xs", name="xs_q")
                    nc.sync.dma_start(
                        xs[:], xqT_r[:, :, qc * 512:(qc + 1) * 512])
                    for ec in range(DC):
                        ps = ppj.tile([P, 512], f32, tag="pj", name="ps_q")
                        for dc in range(DC):
                            nc.tensor.matmul(
                                ps,
                                wq[:, dc, ec * P:(ec + 1) * P],
                                xs[:, dc, :],
                                start=(dc == 0), stop=(dc == DC - 1))
                        nc.vector.tensor_scalar_mul(
                            QT[:, ec, qc * 512:(qc + 1) * 512], ps, 1.0 / 32.0)

            # ---- attention ----
            with tc.tile_pool(name="psc", bufs=2, space="PSUM") as psc, \
                 tc.tile_pool(name="ptr", bufs=2, space="PSUM") as ptr, \
                 tc.tile_pool(name="pav", bufs=4, space="PSUM") as pav, \
                 tc.tile_pool(name="obp", bufs=2) as obp:

                # scores + exp + transpose, slot by slot (V-independent)
                for s in range(len(CAPS)):
                    L = CAPS[s] * P
                    off = offs[s]
                    widths = [512] * (L // 512) + ([256] if L % 512 else [])
                    koff = 0
                    for ci, w in enumerate(widths):
                        ps = psc.tile([P, 512], f32, tag="psc", name=f"sc{s}")
                        for ec in range(DC):
                            nc.tensor.matmul(
                                ps[:, :w],
                                QT[:, ec, s * P:(s + 1) * P],
                                KT[:, ec, koff:koff + w],
                                start=(ec == 0), stop=(ec == DC - 1))
                        koff += w
                        if koff == L:  # apply causal mask to last 256 cols
                            nc.vector.tensor_add(
                                ps[:, w - 256:w], ps[:, w - 256:w], mask_sb[:])
                        nc.scalar.activation(
                            SC[:, off + koff - w:off + koff], ps[:, :w],
                            mybir.ActivationFunctionType.Exp,
                            accum_out=ST[:, s, ci:ci + 1])
                    # rowsum = sum of chunk accumulators; reciprocal
                    nc.vector.tensor_reduce(
                        ST[:, s, 4:5], ST[:, s, 0:len(widths)],
                        axis=mybir.AxisListType.X, op=mybir.AluOpType.add)
                    nc.vector.reciprocal(ST[:, s, 5:6], ST[:, s, 4:5])
                    for j in range(CAPS[s]):
                        pt = ptr.tile([P, P], bf16, tag="ptr", name=f"pt{s}")
                        nc.tensor.transpose(
                            pt, SC[:, off + j * P:off + (j + 1) * P], ident)
                        nc.vector.tensor_copy(
                            WT[:, off + j * P:off + (j + 1) * P], pt)

                # AV, slot by slot (first consumer of V)
                for s in range(len(CAPS)):
                    off = offs[s]
                    avs = [pav.tile([P, 512], f32, tag="pav",
                                    name=f"av{s}_{h}") for h in range(2)]
                    for j in range(CAPS[s]):
                        for h in range(2):
                            nc.tensor.matmul(
                                avs[h],
                                WT[:, off + j * P:off + (j + 1) * P],
                                V[:, j, h * 512:(h + 1) * 512],
                                start=(j == 0), stop=(j == CAPS[s] - 1))
                    ob = obp.tile([P, D], f32, tag="ob", name=f"ob{s}")
                    for h in range(2):
                        nc.vector.tensor_scalar_mul(
                            ob[:, h * 512:(h + 1) * 512], avs[h], ST[:, s, 5:6])
                    nc.sync.dma_start(out_d[s * P:(s + 1) * P, :], ob)

    _split_multi_waits(nc)
    return nc


def _host_prep(x, Wq, Wk, Wv):
    """Build per-core input maps."""
    import ml_dtypes
    bf16 = ml_dtypes.bfloat16

    x = np.ascontiguousarray(x, dtype=np.float32)
    tri = np.where(
        np.arange(P)[None, :] <= np.arange(P)[:, None], 0.0, NEG
    ).astype(np.float32)
    mask_even = np.concatenate(  # parity 0: diag block then fully-masked block
        [tri, np.full((P, P), NEG, np.float32)], axis=1)
    mask_odd = np.concatenate(  # parity 1: fully-visible block then diag block
        [np.zeros((P, P), np.float32), tri], axis=1)

    xb = [np.ascontiguousarray(x[bi].T).astype(bf16) for bi in range(B)]
    wq_b = np.ascontiguousarray(Wq, dtype=np.float32).astype(bf16)
    wk_b = np.ascontiguousarray(Wk, dtype=np.float32).astype(bf16)
    wv_b = np.ascontiguousarray(Wv, dtype=np.float32).astype(bf16)

    in_maps = []
    for c in range(NCORES):
        bi, r = c // 2, c % 2
        rbs = [s - 2 + r for s in CAPS]
        xq = np.concatenate([x[bi, rb * P:(rb + 1) * P, :] for rb in rbs],
                            axis=0)
        in_maps.append({
            "xT": xb[bi],
            "xqT": np.ascontiguousarray(xq.T).astype(bf16),
            "wq": wq_b,
            "wkh": np.ascontiguousarray(wk_b[:, r * EH:(r + 1) * EH]),
            "wvh": np.ascontiguousarray(wv_b[:, r * EH:(r + 1) * EH]),
            "mask": mask_odd if r else mask_even,
        })
    return in_maps


def _host_gather(results):
    out = np.empty((B, N, D), dtype=np.float32)
    for c in range(NCORES):
        bi, r = c // 2, c % 2
        res = results[c]["out"]
        for k, s in enumerate(CAPS):
            rb = s - 2 + r
            out[bi, rb * P:(rb + 1) * P, :] = res[k * P:(k + 1) * P, :]
    return out


def kernel(x, Wq, Wk, Wv, _trace=False, _trace_kwargs=None):
    from concourse.bass_utils import run_bass_kernel_spmd

    if "prog" not in _prog_cache:
        _prog_cache["prog"] = _build_program()
    nc = _prog_cache["prog"]

    in_maps = _host_prep(x, Wq, Wk, Wv)
    kw = dict(_trace_kwargs or {})
    res = run_bass_kernel_spmd(nc, in_maps, list(range(NCORES)),
                               trace=_trace, **kw)
    out = _host_gather(res.results)
    if _trace:
        return out, res
    return out
